# revision 1
# baseline (speedup 1.0000x reference)
"""GCN+JumpingKnowledge inference kernel for Trainium2 (8 NeuronCores).

Computation (matches PyG GCNConv defaults, eval mode):
    deg[v]  = in_degree(v) + 1  (self loops)
    dis     = deg ** -0.5
    agg(x)  = for each v: sum over edges (u->v) incl self loop of
              dis[u]*dis[v] * (x @ W)[u]
    x1 = relu(BN(agg1(node_feat) + b1))
    x2 = agg2(x1) + b2
    out = max(x1, x2) @ Wf + bf

Distribution strategy (8 cores):
  * Destination nodes sharded contiguously across cores (12544 per core,
    graph padded to 100352 nodes with isolated nodes).
  * Layer-1 dense transform h1' = dis * (node_feat @ W1*A) is computed
    replicated on every core (cheaper than an AllGather of it).
  * Layer-2 table h2' = dis * (x1 @ W2) is computed per-shard and
    exchanged with a single AllGather collective.
  * Edge aggregation: edges sorted by (dest-group, source-chunk); source
    rows fetched with dma_gather (int16 indices -> 4 source chunks of
    <=32768 rows); per 128-edge tile a selection matrix S[e,d] =
    (dloc[e] == d) is built on DVE/ACT and the segment sum is done as
    S.T @ msg on the PE into one PSUM bank per 128-dest group.
  * Self loops are appended to the edge list (dis_v * h'_v term).
"""

import math
import os
from contextlib import ExitStack

import numpy as np

# ---------------- problem constants (hardcoded by contract) ----------------
N = 100000
E = 1600000
FIN = 128
HID = 128
FOUT = 40
BN_EPS = 1e-5
NCORES = 8


class Config:
    """Geometry of the kernel; small configs used for simulator tests."""

    def __init__(self, n=N, ncores=NCORES, groups_per_core=98, wave=6,
                 chunk=32768, msg_bf16=False):
        self.n = n
        self.ncores = ncores
        self.G = groups_per_core          # 128-dest groups per core
        self.shard = 128 * groups_per_core
        self.npad = self.shard * ncores
        self.wave = wave                  # groups aggregated per psum wave
        self.chunk = chunk                # max rows addressable by int16 idx
        self.msg_bf16 = msg_bf16
        # chunk layout: assignment ranges + gather base rows.  Row u is
        # assigned to range [starts[i], starts[i+1]); its gather index is
        # u - bases[i], which must stay in [0, chunk).
        starts = []
        bases = []
        s = 0
        while True:
            starts.append(s)
            if self.npad - s <= chunk:           # final chunk covers tail
                bases.append(max(0, min(s, self.npad - chunk)))
                break
            bases.append(s)
            s += chunk
        self.chunk_starts = starts                # range starts
        self.chunk_bases = bases                  # gather AP base row
        self.nchunks = len(starts)
        self.chunk_rows = [min(chunk, self.npad - b) for b in bases]

    def chunk_of(self, u):
        """Chunk id for each (array of) source row id."""
        return np.minimum(np.searchsorted(self.chunk_starts, u, side="right")
                          - 1, self.nchunks - 1)


CFG = Config()


# ---------------------------- host preprocessing ---------------------------

class Sched:
    """Program schedule shared by every core (max over per-core needs).

    Slot layout: per (wave, chunk) segment, the runs of the wave's groups
    are laid out back to back, each sized to the cross-core max count
    M[g,k]; only the segment total is rounded up to 128.  A 128-slot tile
    may therefore span several groups; each (tile, group) pair becomes a
    "subtile" with its own masked dloc column and matmul.
    """

    def __init__(self, cfg, M):
        self.cfg = cfg
        self.M = M  # [G, nchunks] max slot count per (group, chunk)
        g_per_wave = cfg.wave
        self.waves = [list(range(w, min(w + g_per_wave, cfg.G)))
                      for w in range(0, cfg.G, g_per_wave)]
        # segments in program order
        self.segs = []      # (w, k, slot_off, n_slots, subtiles)
        # run_bounds[(g, k)] = (abs_lo, abs_hi) slot interval of the run
        self.run_bounds = {}
        self.tot_slots = 0
        self.tot_tiles = 0
        grp_subtiles = {g: [] for g in range(cfg.G)}
        for w, groups in enumerate(self.waves):
            for k in range(cfg.nchunks):
                seg_used = int(M[groups, k].sum())
                if seg_used == 0:
                    continue
                n_slots = -(-seg_used // 128) * 128
                off = self.tot_slots
                pos = off
                for g in groups:
                    if M[g, k]:
                        self.run_bounds[(g, k)] = (pos, pos + int(M[g, k]))
                        pos += int(M[g, k])
                subtiles = []
                for lt in range(n_slots // 128):
                    t_lo, t_hi = off + lt * 128, off + (lt + 1) * 128
                    for g in groups:
                        b = self.run_bounds.get((g, k))
                        if b and b[0] < t_hi and b[1] > t_lo:
                            subtiles.append([lt, g, False, False])
                            grp_subtiles[g].append(subtiles[-1])
                self.segs.append((w, k, off, n_slots, subtiles))
                self.tot_slots += n_slots
                self.tot_tiles += len(subtiles)
        for g in range(cfg.G):
            sts = grp_subtiles[g]
            assert sts, f"group {g} has no subtiles"
            sts[0][2] = True     # first
            sts[-1][3] = True    # last


def _edge_buckets(cfg, row, col, core):
    """Per-core edge lists (with self loops) keyed by (group, chunk)."""
    lo, hi = core * cfg.shard, (core + 1) * cfg.shard
    m = (col >= lo) & (col < hi)
    r, c = row[m], col[m]
    # self loops for real nodes of this shard
    sl = np.arange(lo, min(hi, cfg.n), dtype=np.int64)
    r = np.concatenate([r, sl])
    c = np.concatenate([c, sl])
    g = (c - lo) >> 7
    k = cfg.chunk_of(r)
    lidx = r - np.asarray(cfg.chunk_bases, np.int64)[k]
    dloc = (c - lo) & 127
    order = np.lexsort((k, g))
    return g[order], k[order], lidx[order], dloc[order]


def prepare(cfg, edge_index):
    """Build the shared schedule + per-core idx/dloc streams."""
    row = np.asarray(edge_index[0], np.int64)
    col = np.asarray(edge_index[1], np.int64)
    deg = np.bincount(col, minlength=cfg.n).astype(np.float32) + 1.0
    dis = 1.0 / np.sqrt(deg)
    dis_pad = np.zeros(cfg.npad, np.float32)
    dis_pad[:cfg.n] = dis

    per_core = [_edge_buckets(cfg, row, col, c) for c in range(cfg.ncores)]
    counts = np.zeros((cfg.ncores, cfg.G, cfg.nchunks), np.int64)
    for c, (g, k, _, _) in enumerate(per_core):
        np.add.at(counts[c], (g, k), 1)
    M = counts.max(axis=0)  # [G, nchunks] run lengths
    sched = Sched(cfg, M)

    # slot -> owning group map (uniform across cores)
    slot_group = np.full(sched.tot_slots, -1, np.int64)
    for (g, k), (lo, hi) in sched.run_bounds.items():
        slot_group[lo:hi] = g

    idx_all = []
    dloc_all = []
    for c in range(cfg.ncores):
        g, k, lidx, dloc = per_core[c]
        slots_idx = np.zeros(sched.tot_slots, np.int32)
        slots_dloc = np.full(sched.tot_slots, -1.0, np.float32)
        key = g * cfg.nchunks + k
        bounds = np.searchsorted(key, np.arange(cfg.G * cfg.nchunks + 1))
        for (gg, kk), (lo, hi) in sched.run_bounds.items():
            b0, b1 = bounds[gg * cfg.nchunks + kk], bounds[gg * cfg.nchunks + kk + 1]
            n = b1 - b0
            assert n <= hi - lo
            slots_idx[lo:lo + n] = lidx[b0:b1]
            slots_dloc[lo:lo + n] = dloc[b0:b1]
        # wrap idx per segment: position i -> [i%16, i//16], tiled to 128 rows
        idx_w = np.zeros((128, sched.tot_slots // 16), np.int16)
        for w, kk, seg_off, n_slots, subtiles in sched.segs:
            seg = slots_idx[seg_off:seg_off + n_slots]
            wrapped = seg.reshape(-1, 16).T.astype(np.int16)   # [16, n/16]
            idx_w[:, seg_off // 16:(seg_off + n_slots) // 16] = np.tile(
                wrapped, (8, 1))
        # per-subtile dloc columns, masked to the subtile's group
        dloc_w = np.full((128, sched.tot_tiles), -1.0, np.float32)
        tid = 0
        for w, kk, seg_off, n_slots, subtiles in sched.segs:
            for lt, gg, first, last in subtiles:
                t_lo = seg_off + lt * 128
                sl = slice(t_lo, t_lo + 128)
                dloc_w[:, tid] = np.where(slot_group[sl] == gg,
                                          slots_dloc[sl], -1.0)
                tid += 1
        idx_all.append(idx_w)
        dloc_all.append(np.ascontiguousarray(dloc_w))
    return sched, dis_pad, idx_all, dloc_all


# ------------------------------- bass builder ------------------------------

def build_module(cfg, sched, reps=1, single=False, ag_copy=False):
    import concourse.bacc as bacc
    import concourse.tile as tile
    from concourse import mybir
    import concourse.bass as bass

    f32 = mybir.dt.float32
    mdt = mybir.dt.bfloat16 if cfg.msg_bf16 else f32
    i16 = mybir.dt.int16
    eq = mybir.AluOpType.is_equal
    mult = mybir.AluOpType.mult
    add = mybir.AluOpType.add
    amax = mybir.AluOpType.max
    Act = mybir.ActivationFunctionType

    nc = bacc.Bacc("TRN2", target_bir_lowering=False, debug=False,
                   num_devices=1 if single else cfg.ncores)
    npad, shard, G = cfg.npad, cfg.shard, cfg.G
    ntiles_all = npad // 128
    tot16 = sched.tot_slots // 16
    TT = sched.tot_tiles

    # --- I/O ---
    ein, eout = "ExternalInput", "ExternalOutput"
    nfT_d = nc.dram_tensor("nfT", [128, npad], mdt, kind=ein)
    idx_d = nc.dram_tensor("idxs", [128, tot16], i16, kind=ein)
    dloc_d = nc.dram_tensor("dloc", [128, TT], f32, kind=ein)
    diss_d = nc.dram_tensor("diss", [128, G], f32, kind=ein)
    w1a_d = nc.dram_tensor("w1a", [FIN, HID], mdt, kind=ein)
    w2_d = nc.dram_tensor("w2", [HID, HID], f32, kind=ein)
    wf_d = nc.dram_tensor("wf", [HID, FOUT], f32, kind=ein)
    d1_d = nc.dram_tensor("d1t", [128, HID], f32, kind=ein)
    b2_d = nc.dram_tensor("b2t", [128, HID], f32, kind=ein)
    bf_d = nc.dram_tensor("bft", [128, FOUT], f32, kind=ein)
    iota_d = nc.dram_tensor("iota", [128, 128], mdt, kind=ein)
    ident_d = nc.dram_tensor("ident", [128, 128], f32, kind=ein)
    out_d = nc.dram_tensor("out", [shard, FOUT], f32, kind=eout)
    # internal
    h1_d = nc.dram_tensor("h1full", [npad, HID], mdt)
    h2s_d = nc.dram_tensor("h2shard", [shard, HID], mdt)
    h2f_d = nc.dram_tensor("h2full", [npad, HID], mdt, addr_space="Shared")

    with tile.TileContext(nc) as tc, ExitStack() as ctx:
        from concourse.library_config import mlp as mlp_lib
        nc.gpsimd.load_library(mlp_lib)

        consts = ctx.enter_context(tc.tile_pool(name="consts", bufs=1))
        psum = ctx.enter_context(tc.tile_pool(name="psum", bufs=8,
                                              space="PSUM"))
        xpool = ctx.enter_context(tc.tile_pool(name="x", bufs=4))
        hpool = ctx.enter_context(tc.tile_pool(name="h", bufs=4))
        idxp = ctx.enter_context(tc.tile_pool(name="idx", bufs=3))
        msgp = ctx.enter_context(tc.tile_pool(name="msg", bufs=3))
        spool = ctx.enter_context(tc.tile_pool(name="s", bufs=6))
        x1pool = ctx.enter_context(tc.tile_pool(name="x1", bufs=G))
        epool = ctx.enter_context(tc.tile_pool(name="ep", bufs=3))

        def load_const(dram, shape, dtype):
            t = consts.tile(shape, dtype, tag=dram.name)
            nc.sync.dma_start(t[:], dram.ap())
            return t

        w1a = load_const(w1a_d, [FIN, HID], mdt)
        w2 = load_const(w2_d, [HID, HID], f32)
        wf = load_const(wf_d, [HID, FOUT], f32)
        d1t = load_const(d1_d, [128, HID], f32)
        b2t = load_const(b2_d, [128, HID], f32)
        bft = load_const(bf_d, [128, FOUT], f32)
        iota = load_const(iota_d, [128, 128], mdt)
        ident = load_const(ident_d, [128, 128], f32)
        diss = load_const(diss_d, [128, G], f32)
        dloc = load_const(dloc_d, [128, TT], f32)

        for rep in range(reps):
            # ---------------- phase A: h1' for all nodes (replicated) ----------
            # nfT comes pre-scaled by dis from the host, so h1' is a plain
            # matmul.  Process 1024 nodes per DMA, 512 per psum bank.
            BW = 1024
            for j0 in range(0, ntiles_all * 128, BW):
                bw = min(BW, ntiles_all * 128 - j0)
                nf = xpool.tile([128, BW], mdt, tag="nf")
                nc.sync.dma_start(nf[:, :bw], nfT_d[:, j0:j0 + bw])
                for s0 in range(0, bw, 512):
                    sw = min(512, bw - s0)
                    ps = psum.tile([128, 512], f32, tag="ps")
                    for m0 in range(0, sw, 128):
                        nc.tensor.matmul(
                            ps[:, m0:m0 + 128],
                            lhsT=nf[:, s0 + m0:s0 + m0 + 128],
                            rhs=w1a[:], start=True, stop=True)
                    h = hpool.tile([128, 4, HID], mdt, tag="h")
                    nc.scalar.activation(h[:, :sw // 128, :], ps[:, :sw],
                                         Act.Copy)
                    v0 = j0 + s0
                    nc.sync.dma_start(
                        h1_d[v0:v0 + sw, :].rearrange(
                            "(j p) f -> p j f", p=128),
                        h[:, :sw // 128, :])

            x1_tiles = [x1pool.tile([128, HID], f32, tag="x1", name=f"x1_{rep}_{g}")
                        for g in range(G)]

            # ---------------- edge aggregation -------------------------------
            sbuild_ctr = [0]

            def build_s(s_ap, tid):
                # S[e, d] = (dloc[e] == d); 3:2 split between DVE and ACT
                sbuild_ctr[0] += 1
                if sbuild_ctr[0] % 5 < 4:
                    nc.vector.tensor_scalar(s_ap, iota[:], dloc[:, tid:tid + 1],
                                            None, op0=eq)
                else:
                    tmp = spool.tile([128, 128], mdt, tag="stmp")
                    nc.scalar.activation(tmp[:], iota[:], Act.Square,
                                         bias=dloc[:, tid:tid + 1], scale=-1.0)
                    nc.scalar.activation(s_ap, tmp[:], Act.Relu,
                                         bias=1.0, scale=-1.0)

            def edge_phase(table_d, epilogue):
                tid0 = [0]
                for w, groups in enumerate(sched.waves):
                    gps = {g: psum.tile([128, HID], f32, tag="ps",
                                        name=f"agg_{rep}_w{w}_g{g}")
                           for g in groups}
                    wave_segs = [s for s in sched.segs if s[0] == w]
                    w_off = wave_segs[0][2]
                    w_slots = sum(s[3] for s in wave_segs)
                    idx_sb = idxp.tile([128, w_slots // 16], i16, tag="idx")
                    nc.sync.dma_start(
                        idx_sb[:],
                        idx_d[:, w_off // 16:(w_off + w_slots) // 16])
                    for (sw, k, seg_off, n_slots, subtiles) in wave_segs:
                        so16 = (seg_off - w_off) // 16
                        J = n_slots // 128
                        msg = msgp.tile([128, J, 128], mdt, tag="msg")
                        base = cfg.chunk_bases[k]
                        # HW SWDGE ring caps a single gather somewhere between
                        # 1024 and 2048 descriptors -> split large segments.
                        GMAX = 1024
                        for sub in range(0, n_slots, GMAX):
                            ns = min(GMAX, n_slots - sub)
                            nc.gpsimd.dma_gather(
                                out_ap=msg[:, sub // 128:(sub + ns) // 128, :],
                                in_ap=table_d[base:base + cfg.chunk_rows[k], :],
                                idxs_ap=idx_sb[:, so16 + sub // 16:
                                               so16 + (sub + ns) // 16],
                                num_idxs=ns,
                                num_idxs_reg=ns,
                                elem_size=HID,
                            )
                        for lt, g, first, last in subtiles:
                            s = spool.tile([128, 128], mdt, tag="s")
                            build_s(s[:], tid0[0])
                            tid0[0] += 1
                            nc.tensor.matmul(gps[g][:], lhsT=s[:],
                                             rhs=msg[:, lt, :],
                                             start=first, stop=last)
                    for g in groups:
                        epilogue(g, gps[g])

            # ---------------- layer 1 ----------------------------------------
            def epilogue1(g, ps):
                x1 = x1_tiles[g]
                # x1 = relu(dis*psum + D1)
                nc.vector.tensor_scalar(x1[:], ps[:], diss[:, g:g + 1], None,
                                        op0=mult)
                nc.vector.tensor_tensor(x1[:], x1[:], d1t[:], op=add)
                nc.scalar.activation(x1[:], x1[:], Act.Relu)
                # h2' = dis * (x1 @ W2); via PE transpose of x1
                pt = psum.tile([128, 128], f32, tag="ps")
                nc.tensor.transpose(pt[:], x1[:], ident[:])
                x1t = epool.tile([128, 128], f32, tag="x1t")
                nc.vector.tensor_copy(x1t[:], pt[:])
                ph = psum.tile([128, HID], f32, tag="ps")
                nc.tensor.matmul(ph[:], lhsT=x1t[:], rhs=w2[:], start=True,
                                 stop=True)
                h2 = epool.tile([128, HID], mdt, tag="h2")
                nc.scalar.activation(h2[:], ph[:], Act.Copy,
                                     scale=diss[:, g:g + 1])
                nc.sync.dma_start(h2s_d[g * 128:(g + 1) * 128, :], h2[:])

            edge_phase(h1_d, epilogue1)

            # ---------------- exchange layer-2 table -------------------------
            if single or ag_copy:
                # stand-in for the AllGather so TimelineSim (single core,
                # no collectives) can run; timing of the real AG is separate
                nc.sync.dma_start(h2f_d[0:shard, :], h2s_d.ap())
            else:
                nc.gpsimd.collective_compute(
                    "AllGather",
                    mybir.AluOpType.bypass,
                    replica_groups=[list(range(cfg.ncores))],
                    ins=[h2s_d.ap()],
                    outs=[h2f_d.ap()],
                )

            # ---------------- layer 2 + JK + final ---------------------------
            def epilogue2(g, ps):
                x2 = epool.tile([128, HID], f32, tag="x2")
                nc.vector.tensor_scalar(x2[:], ps[:], diss[:, g:g + 1], None,
                                        op0=mult)
                nc.vector.tensor_tensor(x2[:], x2[:], b2t[:], op=add)
                nc.vector.tensor_tensor(x2[:], x2[:], x1_tiles[g][:], op=amax)
                pt = psum.tile([128, 128], f32, tag="ps")
                nc.tensor.transpose(pt[:], x2[:], ident[:])
                xt = epool.tile([128, 128], f32, tag="xt")
                nc.vector.tensor_copy(xt[:], pt[:])
                po = psum.tile([128, FOUT], f32, tag="ps")
                nc.tensor.matmul(po[:], lhsT=xt[:], rhs=wf[:], start=True,
                                 stop=True)
                ob = epool.tile([128, FOUT], f32, tag="ob")
                nc.vector.tensor_tensor(ob[:], po[:], bft[:], op=add)
                nc.sync.dma_start(out_d[g * 128:(g + 1) * 128, :], ob[:])

            edge_phase(h2f_d, epilogue2)

    nc.compile()
    return nc


# ------------------------------- host driver -------------------------------

def make_in_maps(cfg, sched, inputs, dis_pad, idx_all, dloc_all):
    node_feat = np.asarray(inputs["node_feat"], np.float32)
    W1 = np.asarray(inputs["W1"], np.float32)
    b1 = np.asarray(inputs["b1"], np.float32)
    gamma1 = np.asarray(inputs["gamma1"], np.float32)
    beta1 = np.asarray(inputs["beta1"], np.float32)
    mean1 = np.asarray(inputs["mean1"], np.float32)
    var1 = np.asarray(inputs["var1"], np.float32)
    W2 = np.asarray(inputs["W2"], np.float32)
    b2 = np.asarray(inputs["b2"], np.float32)
    Wf = np.asarray(inputs["Wf"], np.float32)
    bf = np.asarray(inputs["bf"], np.float32)

    A = gamma1 / np.sqrt(var1 + BN_EPS)
    W1A = (W1 * A[None, :]).astype(np.float32)
    D1 = (b1 * A + beta1 - mean1 * A).astype(np.float32)

    npad = cfg.npad
    nf_pad = np.zeros((npad, FIN), np.float32)
    nf_pad[:cfg.n] = node_feat
    nf_pad *= dis_pad[:, None]          # fold dis_u into the gather table
    nfT = np.ascontiguousarray(nf_pad.T)                     # [128, npad]
    iota = np.tile(np.arange(128, dtype=np.float32), (128, 1))
    mdt_np = np.dtype("bfloat16") if cfg.msg_bf16 else np.float32
    if cfg.msg_bf16:
        import ml_dtypes
        mdt_np = ml_dtypes.bfloat16

    common = {
        "nfT": nfT.astype(mdt_np),
        "w1a": W1A.astype(mdt_np),
        "w2": W2,
        "wf": Wf,
        "d1t": np.tile(D1, (128, 1)),
        "b2t": np.tile(b2, (128, 1)),
        "bft": np.tile(bf, (128, 1)),
        "iota": iota.astype(mdt_np),
        "ident": np.eye(128, dtype=np.float32),
    }
    in_maps = []
    for c in range(cfg.ncores):
        lo = c * cfg.shard
        diss = dis_pad[lo:lo + cfg.shard].reshape(-1, 128).T
        in_maps.append(dict(
            common,
            idxs=idx_all[c],
            dloc=dloc_all[c],
            diss=np.ascontiguousarray(diss),
        ))
    return in_maps


def run(cfg, inputs, trace=False, verbose=False):
    """Full pipeline: prep -> build -> execute on 8 cores -> assemble."""
    import time
    from concourse.bass_utils import run_bass_kernel_spmd
    from concourse.bass_interp import get_hw_module

    t0 = time.time()
    sched, dis_pad, idx_all, dloc_all = prepare(cfg, inputs["edge_index"])
    if verbose:
        print(f"[prep {time.time()-t0:.1f}s] tiles={sched.tot_tiles} "
              f"slots={sched.tot_slots}", flush=True)
    t0 = time.time()
    nc = build_module(cfg, sched)
    if verbose:
        print(f"[build+compile {time.time()-t0:.1f}s]", flush=True)
    t0 = time.time()
    in_maps = make_in_maps(cfg, sched, inputs, dis_pad, idx_all, dloc_all)
    nc.m = get_hw_module(nc.m)
    res = run_bass_kernel_spmd(nc, in_maps, core_ids=list(range(cfg.ncores)),
                               trace=trace)
    if verbose:
        print(f"[execute {time.time()-t0:.1f}s]", flush=True)
    out = np.concatenate([r["out"] for r in res.results], axis=0)[:cfg.n]
    return np.asarray(out, np.float32), res


def kernel(**inputs) -> np.ndarray:
    out, _ = run(CFG, inputs, trace=False)
    return out



# revision 12
# speedup vs baseline: 2.0017x; 2.0017x over previous
"""GCN+JumpingKnowledge inference kernel for Trainium2 (8 NeuronCores).

Computation (matches PyG GCNConv defaults, eval mode):
    deg[v]  = in_degree(v) + 1  (self loops)
    dis     = deg ** -0.5
    agg(x)  = for each v: sum over edges (u->v) incl self loop of
              dis[u]*dis[v] * (x @ W)[u]
    x1 = relu(BN(agg1(node_feat) + b1))
    x2 = agg2(x1) + b2
    out = max(x1, x2) @ Wf + bf

Distribution strategy (8 cores):
  * Destination nodes sharded contiguously across cores (12544 per core,
    graph padded to 100352 nodes with isolated nodes).
  * Layer-1 dense transform h1' = dis * (node_feat @ W1*A) is computed
    replicated on every core (cheaper than an AllGather of it).
  * Layer-2 table h2' = dis * (x1 @ W2) is computed per-shard and
    exchanged with a single AllGather collective.
  * Edge aggregation: edges sorted by (dest-group, source-chunk); source
    rows fetched with dma_gather (int16 indices -> 4 source chunks of
    <=32768 rows); per 128-edge tile a selection matrix S[e,d] =
    (dloc[e] == d) is built on DVE/ACT and the segment sum is done as
    S.T @ msg on the PE into one PSUM bank per 128-dest group.
  * Self loops are appended to the edge list (dis_v * h'_v term).
"""

import math
import os
from contextlib import ExitStack

import numpy as np

# ---------------- problem constants (hardcoded by contract) ----------------
N = 100000
E = 1600000
FIN = 128
HID = 128
FOUT = 40
BN_EPS = 1e-5
NCORES = 8


class Config:
    """Geometry of the kernel; small configs used for simulator tests."""

    def __init__(self, n=N, ncores=NCORES, groups_per_core=98, wave=6,
                 chunk=32768, msg_bf16=True):
        self.n = n
        self.ncores = ncores
        self.G = groups_per_core          # 128-dest groups per core
        self.shard = 128 * groups_per_core
        self.npad = self.shard * ncores
        self.wave = wave                  # groups aggregated per psum wave
        self.chunk = chunk                # max rows addressable by int16 idx
        self.msg_bf16 = msg_bf16
        # chunk layout: assignment ranges + gather base rows.  Row u is
        # assigned to range [starts[i], starts[i+1]); its gather index is
        # u - bases[i], which must stay in [0, chunk).
        starts = []
        bases = []
        s = 0
        while True:
            starts.append(s)
            if self.npad - s <= chunk:           # final chunk covers tail
                bases.append(max(0, min(s, self.npad - chunk)))
                break
            bases.append(s)
            s += chunk
        self.chunk_starts = starts                # range starts
        self.chunk_bases = bases                  # gather AP base row
        self.nchunks = len(starts)
        self.chunk_rows = [min(chunk, self.npad - b) for b in bases]

    def chunk_of(self, u):
        """Chunk id for each (array of) source row id."""
        return np.minimum(np.searchsorted(self.chunk_starts, u, side="right")
                          - 1, self.nchunks - 1)


CFG = Config()


# ---------------------------- host preprocessing ---------------------------

class Sched:
    """Program schedule shared by every core (max over per-core needs).

    Slot layout: per (wave, chunk) segment, the runs of the wave's groups
    are laid out back to back, each sized to the cross-core max count
    M[g,k]; only the segment total is rounded up to 128.  A 128-slot tile
    may therefore span several groups; each (tile, group) pair becomes a
    "subtile" with its own masked dloc column and matmul.
    """

    def __init__(self, cfg, M):
        self.cfg = cfg
        self.M = M  # [G, nchunks] max slot count per (group, chunk)
        g_per_wave = cfg.wave
        self.waves = [list(range(w, min(w + g_per_wave, cfg.G)))
                      for w in range(0, cfg.G, g_per_wave)]
        # segments in program order
        self.segs = []      # (w, k, slot_off, n_slots, subtiles)
        # run_bounds[(g, k)] = (abs_lo, abs_hi) slot interval of the run
        self.run_bounds = {}
        self.tot_slots = 0
        self.tot_tiles = 0
        grp_subtiles = {g: [] for g in range(cfg.G)}
        for w, groups in enumerate(self.waves):
            for k in range(cfg.nchunks):
                seg_used = int(M[groups, k].sum())
                if seg_used == 0:
                    continue
                n_slots = -(-seg_used // 128) * 128
                off = self.tot_slots
                pos = off
                for g in groups:
                    if M[g, k]:
                        self.run_bounds[(g, k)] = (pos, pos + int(M[g, k]))
                        pos += int(M[g, k])
                subtiles = []
                for lt in range(n_slots // 128):
                    t_lo, t_hi = off + lt * 128, off + (lt + 1) * 128
                    for g in groups:
                        b = self.run_bounds.get((g, k))
                        if b and b[0] < t_hi and b[1] > t_lo:
                            subtiles.append([lt, g, False, False])
                            grp_subtiles[g].append(subtiles[-1])
                self.segs.append((w, k, off, n_slots, subtiles))
                self.tot_slots += n_slots
                self.tot_tiles += len(subtiles)
        for g in range(cfg.G):
            sts = grp_subtiles[g]
            assert sts, f"group {g} has no subtiles"
            sts[0][2] = True     # first
            sts[-1][3] = True    # last


def _edge_buckets(cfg, row, col, core):
    """Per-core edge lists (with self loops) keyed by (group, chunk)."""
    lo, hi = core * cfg.shard, (core + 1) * cfg.shard
    m = (col >= lo) & (col < hi)
    r, c = row[m], col[m]
    # self loops for real nodes of this shard
    sl = np.arange(lo, min(hi, cfg.n), dtype=np.int64)
    r = np.concatenate([r, sl])
    c = np.concatenate([c, sl])
    g = (c - lo) >> 7
    k = cfg.chunk_of(r)
    lidx = r - np.asarray(cfg.chunk_bases, np.int64)[k]
    dloc = (c - lo) & 127
    order = np.lexsort((k, g))
    return g[order], k[order], lidx[order], dloc[order]


def prepare(cfg, edge_index):
    """Build the shared schedule + per-core idx/dloc streams."""
    row = np.asarray(edge_index[0], np.int64)
    col = np.asarray(edge_index[1], np.int64)
    deg = np.bincount(col, minlength=cfg.n).astype(np.float32) + 1.0
    dis = 1.0 / np.sqrt(deg)
    dis_pad = np.zeros(cfg.npad, np.float32)
    dis_pad[:cfg.n] = dis

    per_core = [_edge_buckets(cfg, row, col, c) for c in range(cfg.ncores)]
    counts = np.zeros((cfg.ncores, cfg.G, cfg.nchunks), np.int64)
    for c, (g, k, _, _) in enumerate(per_core):
        np.add.at(counts[c], (g, k), 1)
    M = counts.max(axis=0)  # [G, nchunks] run lengths
    sched = Sched(cfg, M)

    # slot -> owning group map (uniform across cores)
    slot_group = np.full(sched.tot_slots, -1, np.int64)
    for (g, k), (lo, hi) in sched.run_bounds.items():
        slot_group[lo:hi] = g

    idx_all = []
    dloc_all = []
    for c in range(cfg.ncores):
        g, k, lidx, dloc = per_core[c]
        slots_idx = np.zeros(sched.tot_slots, np.int32)
        slots_dloc = np.full(sched.tot_slots, -1.0, np.float32)
        key = g * cfg.nchunks + k
        bounds = np.searchsorted(key, np.arange(cfg.G * cfg.nchunks + 1))
        for (gg, kk), (lo, hi) in sched.run_bounds.items():
            b0, b1 = bounds[gg * cfg.nchunks + kk], bounds[gg * cfg.nchunks + kk + 1]
            n = b1 - b0
            assert n <= hi - lo
            slots_idx[lo:lo + n] = lidx[b0:b1]
            slots_dloc[lo:lo + n] = dloc[b0:b1]
        # wrap idx per segment: position i -> [i%16, i//16], tiled to 128 rows
        idx_w = np.zeros((128, sched.tot_slots // 16), np.int16)
        for w, kk, seg_off, n_slots, subtiles in sched.segs:
            seg = slots_idx[seg_off:seg_off + n_slots]
            wrapped = seg.reshape(-1, 16).T.astype(np.int16)   # [16, n/16]
            idx_w[:, seg_off // 16:(seg_off + n_slots) // 16] = np.tile(
                wrapped, (8, 1))
        # per-subtile dloc columns, masked to the subtile's group
        dloc_w = np.full((128, sched.tot_tiles), -1.0, np.float32)
        tid = 0
        for w, kk, seg_off, n_slots, subtiles in sched.segs:
            for lt, gg, first, last in subtiles:
                t_lo = seg_off + lt * 128
                sl = slice(t_lo, t_lo + 128)
                dloc_w[:, tid] = np.where(slot_group[sl] == gg,
                                          slots_dloc[sl], -1.0)
                tid += 1
        idx_all.append(idx_w)
        dloc_all.append(np.ascontiguousarray(dloc_w))
    return sched, dis_pad, idx_all, dloc_all


# ------------------------------- bass builder ------------------------------

def build_module(cfg, sched, reps=1, single=False, ag_copy=False):
    import concourse.bacc as bacc
    import concourse.tile as tile
    from concourse import mybir
    import concourse.bass as bass

    f32 = mybir.dt.float32
    mdt = mybir.dt.bfloat16 if cfg.msg_bf16 else f32
    i16 = mybir.dt.int16
    eq = mybir.AluOpType.is_equal
    mult = mybir.AluOpType.mult
    add = mybir.AluOpType.add
    amax = mybir.AluOpType.max
    Act = mybir.ActivationFunctionType

    nc = bacc.Bacc("TRN2", target_bir_lowering=False, debug=False,
                   num_devices=1 if single else cfg.ncores,
                   num_swdge_queues=4)
    npad, shard, G = cfg.npad, cfg.shard, cfg.G
    ntiles_all = npad // 128
    tot16 = sched.tot_slots // 16
    TT = sched.tot_tiles

    # --- I/O ---
    ein, eout = "ExternalInput", "ExternalOutput"
    nfT_d = nc.dram_tensor("nfT", [128, npad], mdt, kind=ein)
    idx_d = nc.dram_tensor("idxs", [128, tot16], i16, kind=ein)
    dloc_d = nc.dram_tensor("dloc", [128, TT], mdt, kind=ein)
    diss_d = nc.dram_tensor("diss", [128, G], f32, kind=ein)
    w1a_d = nc.dram_tensor("w1a", [FIN, HID], mdt, kind=ein)
    w2_d = nc.dram_tensor("w2", [HID, HID], mdt, kind=ein)
    wf_d = nc.dram_tensor("wf", [HID, FOUT], mdt, kind=ein)
    d1_d = nc.dram_tensor("d1t", [128, HID], f32, kind=ein)
    b2_d = nc.dram_tensor("b2t", [128, HID], f32, kind=ein)
    bf_d = nc.dram_tensor("bft", [128, FOUT], f32, kind=ein)
    iota_d = nc.dram_tensor("iota", [128, 128], mdt, kind=ein)
    ident_d = nc.dram_tensor("ident", [128, 128], f32, kind=ein)
    out_d = nc.dram_tensor("out", [shard, FOUT], f32, kind=eout)
    # internal
    h1_d = nc.dram_tensor("h1full", [npad, HID], mdt)
    h2s_d = nc.dram_tensor("h2shard", [shard, HID], mdt)
    h2f_d = nc.dram_tensor("h2full", [npad, HID], mdt, addr_space="Shared")

    with tile.TileContext(nc) as tc, ExitStack() as ctx:
        from concourse.library_config import mlp as mlp_lib
        nc.gpsimd.load_library(mlp_lib)

        consts = ctx.enter_context(tc.tile_pool(name="consts", bufs=1))
        psum = ctx.enter_context(tc.tile_pool(name="psum", bufs=8,
                                              space="PSUM"))
        xpool = ctx.enter_context(tc.tile_pool(name="x", bufs=4))
        hpool = ctx.enter_context(tc.tile_pool(name="h", bufs=4))
        idxp = ctx.enter_context(tc.tile_pool(name="idx", bufs=3))
        msgp = ctx.enter_context(tc.tile_pool(name="msg", bufs=3))
        spool = ctx.enter_context(tc.tile_pool(name="s", bufs=3))
        x1pool = ctx.enter_context(tc.tile_pool(name="x1", bufs=G))
        epool = ctx.enter_context(tc.tile_pool(name="ep", bufs=3))

        def load_const(dram, shape, dtype):
            t = consts.tile(shape, dtype, tag=dram.name)
            nc.sync.dma_start(t[:], dram.ap())
            return t

        w1a = load_const(w1a_d, [FIN, HID], mdt)
        w2 = load_const(w2_d, [HID, HID], mdt)
        wf = load_const(wf_d, [HID, FOUT], mdt)
        d1t = load_const(d1_d, [128, HID], f32)
        b2t = load_const(b2_d, [128, HID], f32)
        bft = load_const(bf_d, [128, FOUT], f32)
        iota = load_const(iota_d, [128, 128], mdt)
        ident = load_const(ident_d, [128, 128], f32)
        diss = load_const(diss_d, [128, G], f32)
        dloc = load_const(dloc_d, [128, TT], mdt)

        for rep in range(reps):
            # ---------------- phase A: h1' for all nodes (replicated) ----------
            # nfT comes pre-scaled by dis from the host, so h1' is a plain
            # matmul.  Process 1024 nodes per DMA, 512 per psum bank.
            BW = 1024
            for j0 in range(0, ntiles_all * 128, BW):
                bw = min(BW, ntiles_all * 128 - j0)
                nf = xpool.tile([128, BW], mdt, tag="nf")
                nc.sync.dma_start(nf[:, :bw], nfT_d[:, j0:j0 + bw])
                for s0 in range(0, bw, 512):
                    sw = min(512, bw - s0)
                    ps = psum.tile([128, 512], f32, tag="ps")
                    for m0 in range(0, sw, 128):
                        nc.tensor.matmul(
                            ps[:, m0:m0 + 128],
                            lhsT=nf[:, s0 + m0:s0 + m0 + 128],
                            rhs=w1a[:], start=True, stop=True)
                    h = hpool.tile([128, 4, HID], mdt, tag="h")
                    nc.scalar.activation(h[:, :sw // 128, :], ps[:, :sw],
                                         Act.Copy)
                    v0 = j0 + s0
                    nc.sync.dma_start(
                        h1_d[v0:v0 + sw, :].rearrange(
                            "(j p) f -> p j f", p=128),
                        h[:, :sw // 128, :])

            x1_tiles = [x1pool.tile([128, HID], f32, tag="x1", name=f"x1_{rep}_{g}")
                        for g in range(G)]

            # ---------------- edge aggregation -------------------------------
            qctr = [0]

            def edge_phase(table_d, epilogue):
                tid0 = [0]
                for w, groups in enumerate(sched.waves):
                    gps = {g: psum.tile([128, HID], f32, tag="ps",
                                        name=f"agg_{rep}_w{w}_g{g}")
                           for g in groups}
                    wave_segs = [s for s in sched.segs if s[0] == w]
                    w_off = wave_segs[0][2]
                    w_slots = sum(s[3] for s in wave_segs)
                    idx_sb = idxp.tile([128, w_slots // 16], i16, tag="idx")
                    nc.sync.dma_start(
                        idx_sb[:],
                        idx_d[:, w_off // 16:(w_off + w_slots) // 16])
                    for (sw, k, seg_off, n_slots, subtiles) in wave_segs:
                        so16 = (seg_off - w_off) // 16
                        J = n_slots // 128
                        msg = msgp.tile([128, J, 128], mdt, tag="msg")
                        base = cfg.chunk_bases[k]
                        # HW SWDGE descriptor ring caps one gather at 1024
                        # descriptors; cycle the 4 SWDGE queues so descriptor
                        # generation runs on all 4 Q7 core pairs in parallel.
                        GMAX = 1024
                        for sub in range(0, n_slots, GMAX):
                            ns = min(GMAX, n_slots - sub)
                            nc.gpsimd.dma_gather(
                                out_ap=msg[:, sub // 128:(sub + ns) // 128, :],
                                in_ap=table_d[base:base + cfg.chunk_rows[k], :],
                                idxs_ap=idx_sb[:, so16 + sub // 16:
                                               so16 + (sub + ns) // 16],
                                num_idxs=ns,
                                num_idxs_reg=ns,
                                elem_size=HID,
                                queue_num=qctr[0] % 4,
                            )
                            qctr[0] += 1
                        # batched S build: S[e, d] = (dloc[e] == iota[d]) for
                        # all of the segment's subtiles in one DVE op.
                        nst = len(subtiles)
                        sb = spool.tile([128, nst, 128], mdt, tag="s",
                                        name=f"s_{rep}_{table_d.name}"
                                             f"_w{w}_k{k}")
                        io_b = iota[:].unsqueeze(1).broadcast_to(
                            [128, nst, 128])
                        dl_b = (dloc[:, tid0[0]:tid0[0] + nst].unsqueeze(2)
                                .broadcast_to([128, nst, 128]))
                        nc.vector.tensor_tensor(sb[:], io_b, dl_b, op=eq)
                        tid0[0] += nst
                        for sj, (lt, g, first, last) in enumerate(subtiles):
                            nc.tensor.matmul(gps[g][:], lhsT=sb[:, sj, :],
                                             rhs=msg[:, lt, :],
                                             start=first, stop=last)
                    for g in groups:
                        epilogue(g, gps[g])

            # ---------------- layer 1 ----------------------------------------
            def epilogue1(g, ps):
                x1 = x1_tiles[g]
                # x1 = relu(dis*psum + D1)
                nc.vector.tensor_scalar(x1[:], ps[:], diss[:, g:g + 1], None,
                                        op0=mult)
                nc.vector.tensor_tensor(x1[:], x1[:], d1t[:], op=add)
                nc.scalar.activation(x1[:], x1[:], Act.Relu)
                # h2' = dis * (x1 @ W2); via PE transpose of x1
                pt = psum.tile([128, 128], f32, tag="ps")
                nc.tensor.transpose(pt[:], x1[:], ident[:])
                x1t = epool.tile([128, 128], mdt, tag="x1t")
                nc.vector.tensor_copy(x1t[:], pt[:])
                ph = psum.tile([128, HID], f32, tag="ps")
                nc.tensor.matmul(ph[:], lhsT=x1t[:], rhs=w2[:], start=True,
                                 stop=True)
                h2 = epool.tile([128, HID], mdt, tag="h2")
                nc.scalar.activation(h2[:], ph[:], Act.Copy,
                                     scale=diss[:, g:g + 1])
                nc.sync.dma_start(h2s_d[g * 128:(g + 1) * 128, :], h2[:])

            edge_phase(h1_d, epilogue1)

            # ---------------- exchange layer-2 table -------------------------
            if single or ag_copy:
                # stand-in for the AllGather so TimelineSim (single core,
                # no collectives) can run; timing of the real AG is separate
                nc.sync.dma_start(h2f_d[0:shard, :], h2s_d.ap())
            else:
                nc.gpsimd.collective_compute(
                    "AllGather",
                    mybir.AluOpType.bypass,
                    replica_groups=[list(range(cfg.ncores))],
                    ins=[h2s_d.ap()],
                    outs=[h2f_d.ap()],
                )

            # ---------------- layer 2 + JK + final ---------------------------
            def epilogue2(g, ps):
                x2 = epool.tile([128, HID], f32, tag="x2")
                nc.vector.tensor_scalar(x2[:], ps[:], diss[:, g:g + 1], None,
                                        op0=mult)
                nc.vector.tensor_tensor(x2[:], x2[:], b2t[:], op=add)
                nc.vector.tensor_tensor(x2[:], x2[:], x1_tiles[g][:], op=amax)
                pt = psum.tile([128, 128], f32, tag="ps")
                nc.tensor.transpose(pt[:], x2[:], ident[:])
                xt = epool.tile([128, 128], mdt, tag="xt")
                nc.vector.tensor_copy(xt[:], pt[:])
                po = psum.tile([128, FOUT], f32, tag="ps")
                nc.tensor.matmul(po[:], lhsT=xt[:], rhs=wf[:], start=True,
                                 stop=True)
                ob = epool.tile([128, FOUT], f32, tag="ob")
                nc.vector.tensor_tensor(ob[:], po[:], bft[:], op=add)
                nc.sync.dma_start(out_d[g * 128:(g + 1) * 128, :], ob[:])

            edge_phase(h2f_d, epilogue2)

    nc.compile()
    return nc


# ------------------------------- host driver -------------------------------

def make_in_maps(cfg, sched, inputs, dis_pad, idx_all, dloc_all):
    node_feat = np.asarray(inputs["node_feat"], np.float32)
    W1 = np.asarray(inputs["W1"], np.float32)
    b1 = np.asarray(inputs["b1"], np.float32)
    gamma1 = np.asarray(inputs["gamma1"], np.float32)
    beta1 = np.asarray(inputs["beta1"], np.float32)
    mean1 = np.asarray(inputs["mean1"], np.float32)
    var1 = np.asarray(inputs["var1"], np.float32)
    W2 = np.asarray(inputs["W2"], np.float32)
    b2 = np.asarray(inputs["b2"], np.float32)
    Wf = np.asarray(inputs["Wf"], np.float32)
    bf = np.asarray(inputs["bf"], np.float32)

    A = gamma1 / np.sqrt(var1 + BN_EPS)
    W1A = (W1 * A[None, :]).astype(np.float32)
    D1 = (b1 * A + beta1 - mean1 * A).astype(np.float32)

    npad = cfg.npad
    nf_pad = np.zeros((npad, FIN), np.float32)
    nf_pad[:cfg.n] = node_feat
    nf_pad *= dis_pad[:, None]          # fold dis_u into the gather table
    nfT = np.ascontiguousarray(nf_pad.T)                     # [128, npad]
    iota = np.tile(np.arange(128, dtype=np.float32), (128, 1))
    if cfg.msg_bf16:
        import ml_dtypes
        mdt_np = ml_dtypes.bfloat16
    else:
        mdt_np = np.float32

    common = {
        "nfT": nfT.astype(mdt_np),
        "w1a": W1A.astype(mdt_np),
        "w2": W2.astype(mdt_np),
        "wf": Wf.astype(mdt_np),
        "d1t": np.tile(D1, (128, 1)),
        "b2t": np.tile(b2, (128, 1)),
        "bft": np.tile(bf, (128, 1)),
        "iota": iota.astype(mdt_np),
        "ident": np.eye(128, dtype=np.float32),
    }
    in_maps = []
    for c in range(cfg.ncores):
        lo = c * cfg.shard
        diss = dis_pad[lo:lo + cfg.shard].reshape(-1, 128).T
        in_maps.append(dict(
            common,
            idxs=idx_all[c],
            dloc=dloc_all[c].astype(mdt_np),
            diss=np.ascontiguousarray(diss),
        ))
    return in_maps


def run(cfg, inputs, trace=False, verbose=False):
    """Full pipeline: prep -> build -> execute on 8 cores -> assemble."""
    import time
    from concourse.bass_utils import run_bass_kernel_spmd
    from concourse.bass_interp import get_hw_module

    t0 = time.time()
    sched, dis_pad, idx_all, dloc_all = prepare(cfg, inputs["edge_index"])
    if verbose:
        print(f"[prep {time.time()-t0:.1f}s] tiles={sched.tot_tiles} "
              f"slots={sched.tot_slots}", flush=True)
    t0 = time.time()
    nc = build_module(cfg, sched)
    if verbose:
        print(f"[build+compile {time.time()-t0:.1f}s]", flush=True)
    t0 = time.time()
    in_maps = make_in_maps(cfg, sched, inputs, dis_pad, idx_all, dloc_all)
    nc.m = get_hw_module(nc.m)
    res = run_bass_kernel_spmd(nc, in_maps, core_ids=list(range(cfg.ncores)),
                               trace=trace)
    if verbose:
        print(f"[execute {time.time()-t0:.1f}s]", flush=True)
    out = np.concatenate([r["out"] for r in res.results], axis=0)[:cfg.n]
    return np.asarray(out, np.float32), res


def kernel(**inputs) -> np.ndarray:
    out, _ = run(CFG, inputs, trace=False)
    return out



# revision 13
# speedup vs baseline: 2.0403x; 1.0193x over previous
"""GCN+JumpingKnowledge inference kernel for Trainium2 (8 NeuronCores).

Computation (matches PyG GCNConv defaults, eval mode):
    deg[v]  = in_degree(v) + 1  (self loops)
    dis     = deg ** -0.5
    agg(x)  = for each v: sum over edges (u->v) incl self loop of
              dis[u]*dis[v] * (x @ W)[u]
    x1 = relu(BN(agg1(node_feat) + b1))
    x2 = agg2(x1) + b2
    out = max(x1, x2) @ Wf + bf

Distribution strategy (8 cores):
  * Destination nodes sharded contiguously across cores (12544 per core,
    graph padded to 100352 nodes with isolated nodes).
  * Layer-1 dense transform h1' = dis * (node_feat @ W1*A) is computed
    replicated on every core (cheaper than an AllGather of it).
  * Layer-2 table h2' = dis * (x1 @ W2) is computed per-shard and
    exchanged with a single AllGather collective.
  * Edge aggregation: edges sorted by (dest-group, source-chunk); source
    rows fetched with dma_gather (int16 indices -> 4 source chunks of
    <=32768 rows); per 128-edge tile a selection matrix S[e,d] =
    (dloc[e] == d) is built on DVE/ACT and the segment sum is done as
    S.T @ msg on the PE into one PSUM bank per 128-dest group.
  * Self loops are appended to the edge list (dis_v * h'_v term).
"""

import math
import os
from contextlib import ExitStack

import numpy as np

# ---------------- problem constants (hardcoded by contract) ----------------
N = 100000
E = 1600000
FIN = 128
HID = 128
FOUT = 40
BN_EPS = 1e-5
NCORES = 8


class Config:
    """Geometry of the kernel; small configs used for simulator tests."""

    def __init__(self, n=N, ncores=NCORES, groups_per_core=98, wave=6,
                 chunk=32768, msg_bf16=True):
        self.n = n
        self.ncores = ncores
        self.G = groups_per_core          # 128-dest groups per core
        self.shard = 128 * groups_per_core
        self.npad = self.shard * ncores
        self.wave = wave                  # groups aggregated per psum wave
        self.chunk = chunk                # max rows addressable by int16 idx
        self.msg_bf16 = msg_bf16
        # chunk layout: assignment ranges + gather base rows.  Row u is
        # assigned to range [starts[i], starts[i+1]); its gather index is
        # u - bases[i], which must stay in [0, chunk).
        starts = []
        bases = []
        s = 0
        while True:
            starts.append(s)
            if self.npad - s <= chunk:           # final chunk covers tail
                bases.append(max(0, min(s, self.npad - chunk)))
                break
            bases.append(s)
            s += chunk
        self.chunk_starts = starts                # range starts
        self.chunk_bases = bases                  # gather AP base row
        self.nchunks = len(starts)
        self.chunk_rows = [min(chunk, self.npad - b) for b in bases]

    def chunk_of(self, u):
        """Chunk id for each (array of) source row id."""
        return np.minimum(np.searchsorted(self.chunk_starts, u, side="right")
                          - 1, self.nchunks - 1)


CFG = Config()


# ---------------------------- host preprocessing ---------------------------

class Sched:
    """Program schedule shared by every core (max over per-core needs).

    Slot layout: per (wave, chunk) segment, the runs of the wave's groups
    are laid out back to back, each sized to the cross-core max count
    M[g,k]; only the segment total is rounded up to 128.  A 128-slot tile
    may therefore span several groups; each (tile, group) pair becomes a
    "subtile" with its own masked dloc column and matmul.
    """

    def __init__(self, cfg, M):
        self.cfg = cfg
        self.M = M  # [G, nchunks] max slot count per (group, chunk)
        g_per_wave = cfg.wave
        self.waves = [list(range(w, min(w + g_per_wave, cfg.G)))
                      for w in range(0, cfg.G, g_per_wave)]
        # segments in program order
        self.segs = []      # (w, k, slot_off, n_slots, subtiles)
        # run_bounds[(g, k)] = (abs_lo, abs_hi) slot interval of the run
        self.run_bounds = {}
        self.tot_slots = 0
        self.tot_tiles = 0
        grp_subtiles = {g: [] for g in range(cfg.G)}
        for w, groups in enumerate(self.waves):
            for k in range(cfg.nchunks):
                seg_used = int(M[groups, k].sum())
                if seg_used == 0:
                    continue
                n_slots = -(-seg_used // 128) * 128
                off = self.tot_slots
                pos = off
                for g in groups:
                    if M[g, k]:
                        self.run_bounds[(g, k)] = (pos, pos + int(M[g, k]))
                        pos += int(M[g, k])
                subtiles = []
                for lt in range(n_slots // 128):
                    t_lo, t_hi = off + lt * 128, off + (lt + 1) * 128
                    for g in groups:
                        b = self.run_bounds.get((g, k))
                        if b and b[0] < t_hi and b[1] > t_lo:
                            subtiles.append([lt, g, False, False])
                            grp_subtiles[g].append(subtiles[-1])
                self.segs.append((w, k, off, n_slots, subtiles))
                self.tot_slots += n_slots
                self.tot_tiles += len(subtiles)
        for g in range(cfg.G):
            sts = grp_subtiles[g]
            assert sts, f"group {g} has no subtiles"
            sts[0][2] = True     # first
            sts[-1][3] = True    # last


def _edge_buckets(cfg, row, col, core):
    """Per-core edge lists (with self loops) keyed by (group, chunk)."""
    lo, hi = core * cfg.shard, (core + 1) * cfg.shard
    m = (col >= lo) & (col < hi)
    r, c = row[m], col[m]
    # self loops for real nodes of this shard
    sl = np.arange(lo, min(hi, cfg.n), dtype=np.int64)
    r = np.concatenate([r, sl])
    c = np.concatenate([c, sl])
    g = (c - lo) >> 7
    k = cfg.chunk_of(r)
    lidx = r - np.asarray(cfg.chunk_bases, np.int64)[k]
    dloc = (c - lo) & 127
    order = np.lexsort((k, g))
    return g[order], k[order], lidx[order], dloc[order]


def prepare(cfg, edge_index):
    """Build the shared schedule + per-core idx/dloc streams."""
    row = np.asarray(edge_index[0], np.int64)
    col = np.asarray(edge_index[1], np.int64)
    deg = np.bincount(col, minlength=cfg.n).astype(np.float32) + 1.0
    dis = 1.0 / np.sqrt(deg)
    dis_pad = np.zeros(cfg.npad, np.float32)
    dis_pad[:cfg.n] = dis

    per_core = [_edge_buckets(cfg, row, col, c) for c in range(cfg.ncores)]
    counts = np.zeros((cfg.ncores, cfg.G, cfg.nchunks), np.int64)
    for c, (g, k, _, _) in enumerate(per_core):
        np.add.at(counts[c], (g, k), 1)
    M = counts.max(axis=0)  # [G, nchunks] run lengths
    sched = Sched(cfg, M)

    # slot -> owning group map (uniform across cores)
    slot_group = np.full(sched.tot_slots, -1, np.int64)
    for (g, k), (lo, hi) in sched.run_bounds.items():
        slot_group[lo:hi] = g

    idx_all = []
    dloc_all = []
    for c in range(cfg.ncores):
        g, k, lidx, dloc = per_core[c]
        slots_idx = np.zeros(sched.tot_slots, np.int32)
        slots_dloc = np.full(sched.tot_slots, -1.0, np.float32)
        key = g * cfg.nchunks + k
        bounds = np.searchsorted(key, np.arange(cfg.G * cfg.nchunks + 1))
        for (gg, kk), (lo, hi) in sched.run_bounds.items():
            b0, b1 = bounds[gg * cfg.nchunks + kk], bounds[gg * cfg.nchunks + kk + 1]
            n = b1 - b0
            assert n <= hi - lo
            slots_idx[lo:lo + n] = lidx[b0:b1]
            slots_dloc[lo:lo + n] = dloc[b0:b1]
        # wrap idx per segment: position i -> [i%16, i//16], tiled to 128 rows
        idx_w = np.zeros((128, sched.tot_slots // 16), np.int16)
        for w, kk, seg_off, n_slots, subtiles in sched.segs:
            seg = slots_idx[seg_off:seg_off + n_slots]
            wrapped = seg.reshape(-1, 16).T.astype(np.int16)   # [16, n/16]
            idx_w[:, seg_off // 16:(seg_off + n_slots) // 16] = np.tile(
                wrapped, (8, 1))
        # per-subtile dloc columns, masked to the subtile's group
        dloc_w = np.full((128, sched.tot_tiles), -1.0, np.float32)
        tid = 0
        for w, kk, seg_off, n_slots, subtiles in sched.segs:
            for lt, gg, first, last in subtiles:
                t_lo = seg_off + lt * 128
                sl = slice(t_lo, t_lo + 128)
                dloc_w[:, tid] = np.where(slot_group[sl] == gg,
                                          slots_dloc[sl], -1.0)
                tid += 1
        idx_all.append(idx_w)
        dloc_all.append(np.ascontiguousarray(dloc_w))
    return sched, dis_pad, idx_all, dloc_all


# ------------------------------- bass builder ------------------------------

def build_module(cfg, sched, reps=1, single=False, ag_copy=False):
    import concourse.bacc as bacc
    import concourse.tile as tile
    from concourse import mybir
    import concourse.bass as bass

    f32 = mybir.dt.float32
    mdt = mybir.dt.bfloat16 if cfg.msg_bf16 else f32
    i16 = mybir.dt.int16
    eq = mybir.AluOpType.is_equal
    mult = mybir.AluOpType.mult
    add = mybir.AluOpType.add
    amax = mybir.AluOpType.max
    Act = mybir.ActivationFunctionType

    nc = bacc.Bacc("TRN2", target_bir_lowering=False, debug=False,
                   num_devices=1 if single else cfg.ncores,
                   num_swdge_queues=4)
    npad, shard, G = cfg.npad, cfg.shard, cfg.G
    ntiles_all = npad // 128
    tot16 = sched.tot_slots // 16
    TT = sched.tot_tiles

    # --- I/O ---
    ein, eout = "ExternalInput", "ExternalOutput"
    nfT_d = nc.dram_tensor("nfT", [128, npad], mdt, kind=ein)
    idx_d = nc.dram_tensor("idxs", [128, tot16], i16, kind=ein)
    dloc_d = nc.dram_tensor("dloc", [128, TT], mdt, kind=ein)
    diss_d = nc.dram_tensor("diss", [128, G], f32, kind=ein)
    w1a_d = nc.dram_tensor("w1a", [FIN, HID], mdt, kind=ein)
    w2_d = nc.dram_tensor("w2", [HID, HID], mdt, kind=ein)
    wf_d = nc.dram_tensor("wf", [HID, FOUT], mdt, kind=ein)
    d1_d = nc.dram_tensor("d1t", [128, HID], f32, kind=ein)
    b2_d = nc.dram_tensor("b2t", [128, HID], f32, kind=ein)
    bf_d = nc.dram_tensor("bft", [128, FOUT], f32, kind=ein)
    iota_d = nc.dram_tensor("iota", [128, 128], mdt, kind=ein)
    ident_d = nc.dram_tensor("ident", [128, 128], f32, kind=ein)
    out_d = nc.dram_tensor("out", [shard, FOUT], f32, kind=eout)
    # internal
    h1_d = nc.dram_tensor("h1full", [npad, HID], mdt)
    h2s_d = nc.dram_tensor("h2shard", [shard, HID], mdt)
    h2f_d = nc.dram_tensor("h2full", [npad, HID], mdt, addr_space="Shared")

    with tile.TileContext(nc) as tc, ExitStack() as ctx:
        from concourse.library_config import mlp as mlp_lib
        nc.gpsimd.load_library(mlp_lib)

        consts = ctx.enter_context(tc.tile_pool(name="consts", bufs=1))
        psum = ctx.enter_context(tc.tile_pool(name="psum", bufs=8,
                                              space="PSUM"))
        xpool = ctx.enter_context(tc.tile_pool(name="x", bufs=4))
        hpool = ctx.enter_context(tc.tile_pool(name="h", bufs=4))
        idxp = ctx.enter_context(tc.tile_pool(name="idx", bufs=4))
        msgp = ctx.enter_context(tc.tile_pool(name="msg", bufs=6))
        spool = ctx.enter_context(tc.tile_pool(name="s", bufs=4))
        x1pool = ctx.enter_context(tc.tile_pool(name="x1", bufs=G))
        epool = ctx.enter_context(tc.tile_pool(name="ep", bufs=3))

        def load_const(dram, shape, dtype):
            t = consts.tile(shape, dtype, tag=dram.name)
            nc.sync.dma_start(t[:], dram.ap())
            return t

        w1a = load_const(w1a_d, [FIN, HID], mdt)
        w2 = load_const(w2_d, [HID, HID], mdt)
        wf = load_const(wf_d, [HID, FOUT], mdt)
        d1t = load_const(d1_d, [128, HID], f32)
        b2t = load_const(b2_d, [128, HID], f32)
        bft = load_const(bf_d, [128, FOUT], f32)
        iota = load_const(iota_d, [128, 128], mdt)
        ident = load_const(ident_d, [128, 128], f32)
        diss = load_const(diss_d, [128, G], f32)
        dloc = load_const(dloc_d, [128, TT], mdt)

        for rep in range(reps):
            # ---------------- phase A: h1' for all nodes (replicated) ----------
            # nfT comes pre-scaled by dis from the host, so h1' is a plain
            # matmul.  Process 1024 nodes per DMA, 512 per psum bank.
            BW = 1024
            for j0 in range(0, ntiles_all * 128, BW):
                bw = min(BW, ntiles_all * 128 - j0)
                nf = xpool.tile([128, BW], mdt, tag="nf")
                nc.sync.dma_start(nf[:, :bw], nfT_d[:, j0:j0 + bw])
                for s0 in range(0, bw, 512):
                    sw = min(512, bw - s0)
                    ps = psum.tile([128, 512], f32, tag="ps")
                    for m0 in range(0, sw, 128):
                        nc.tensor.matmul(
                            ps[:, m0:m0 + 128],
                            lhsT=nf[:, s0 + m0:s0 + m0 + 128],
                            rhs=w1a[:], start=True, stop=True)
                    h = hpool.tile([128, 4, HID], mdt, tag="h")
                    nc.scalar.activation(h[:, :sw // 128, :], ps[:, :sw],
                                         Act.Copy)
                    v0 = j0 + s0
                    nc.sync.dma_start(
                        h1_d[v0:v0 + sw, :].rearrange(
                            "(j p) f -> p j f", p=128),
                        h[:, :sw // 128, :])

            x1_tiles = [x1pool.tile([128, HID], f32, tag="x1", name=f"x1_{rep}_{g}")
                        for g in range(G)]

            # ---------------- edge aggregation -------------------------------
            qctr = [0]

            def edge_phase(table_d, epilogue):
                tid0 = [0]
                for w, groups in enumerate(sched.waves):
                    gps = {g: psum.tile([128, HID], f32, tag="ps",
                                        name=f"agg_{rep}_w{w}_g{g}")
                           for g in groups}
                    wave_segs = [s for s in sched.segs if s[0] == w]
                    w_off = wave_segs[0][2]
                    w_slots = sum(s[3] for s in wave_segs)
                    idx_sb = idxp.tile([128, w_slots // 16], i16, tag="idx")
                    nc.sync.dma_start(
                        idx_sb[:],
                        idx_d[:, w_off // 16:(w_off + w_slots) // 16])
                    for (sw, k, seg_off, n_slots, subtiles) in wave_segs:
                        so16 = (seg_off - w_off) // 16
                        J = n_slots // 128
                        msg = msgp.tile([128, J, 128], mdt, tag="msg")
                        base = cfg.chunk_bases[k]
                        # HW SWDGE descriptor ring caps one gather at 1024
                        # descriptors; cycle the 4 SWDGE queues so descriptor
                        # generation runs on all 4 Q7 core pairs in parallel.
                        GMAX = 1024
                        for sub in range(0, n_slots, GMAX):
                            ns = min(GMAX, n_slots - sub)
                            nc.gpsimd.dma_gather(
                                out_ap=msg[:, sub // 128:(sub + ns) // 128, :],
                                in_ap=table_d[base:base + cfg.chunk_rows[k], :],
                                idxs_ap=idx_sb[:, so16 + sub // 16:
                                               so16 + (sub + ns) // 16],
                                num_idxs=ns,
                                num_idxs_reg=ns,
                                elem_size=HID,
                                queue_num=qctr[0] % 4,
                            )
                            qctr[0] += 1
                        # batched S build: S[e, d] = (dloc[e] == iota[d]) for
                        # all of the segment's subtiles in one DVE op.
                        nst = len(subtiles)
                        sb = spool.tile([128, nst, 128], mdt, tag="s",
                                        name=f"s_{rep}_{table_d.name}"
                                             f"_w{w}_k{k}")
                        io_b = iota[:].unsqueeze(1).broadcast_to(
                            [128, nst, 128])
                        dl_b = (dloc[:, tid0[0]:tid0[0] + nst].unsqueeze(2)
                                .broadcast_to([128, nst, 128]))
                        nc.vector.tensor_tensor(sb[:], io_b, dl_b, op=eq)
                        tid0[0] += nst
                        for sj, (lt, g, first, last) in enumerate(subtiles):
                            nc.tensor.matmul(gps[g][:], lhsT=sb[:, sj, :],
                                             rhs=msg[:, lt, :],
                                             start=first, stop=last)
                    for g in groups:
                        epilogue(g, gps[g])

            # ---------------- layer 1 ----------------------------------------
            def epilogue1(g, ps):
                x1 = x1_tiles[g]
                # x1 = relu(dis*psum + D1)
                nc.vector.tensor_scalar(x1[:], ps[:], diss[:, g:g + 1], None,
                                        op0=mult)
                nc.vector.tensor_tensor(x1[:], x1[:], d1t[:], op=add)
                nc.scalar.activation(x1[:], x1[:], Act.Relu)
                # h2' = dis * (x1 @ W2); via PE transpose of x1
                pt = psum.tile([128, 128], f32, tag="ps")
                nc.tensor.transpose(pt[:], x1[:], ident[:])
                x1t = epool.tile([128, 128], mdt, tag="x1t")
                nc.vector.tensor_copy(x1t[:], pt[:])
                ph = psum.tile([128, HID], f32, tag="ps")
                nc.tensor.matmul(ph[:], lhsT=x1t[:], rhs=w2[:], start=True,
                                 stop=True)
                h2 = epool.tile([128, HID], mdt, tag="h2")
                nc.scalar.activation(h2[:], ph[:], Act.Copy,
                                     scale=diss[:, g:g + 1])
                nc.sync.dma_start(h2s_d[g * 128:(g + 1) * 128, :], h2[:])

            edge_phase(h1_d, epilogue1)

            # ---------------- exchange layer-2 table -------------------------
            if single or ag_copy:
                # stand-in for the AllGather so TimelineSim (single core,
                # no collectives) can run; timing of the real AG is separate
                nc.sync.dma_start(h2f_d[0:shard, :], h2s_d.ap())
            else:
                nc.gpsimd.collective_compute(
                    "AllGather",
                    mybir.AluOpType.bypass,
                    replica_groups=[list(range(cfg.ncores))],
                    ins=[h2s_d.ap()],
                    outs=[h2f_d.ap()],
                )

            # ---------------- layer 2 + JK + final ---------------------------
            def epilogue2(g, ps):
                x2 = epool.tile([128, HID], f32, tag="x2")
                nc.vector.tensor_scalar(x2[:], ps[:], diss[:, g:g + 1], None,
                                        op0=mult)
                nc.vector.tensor_tensor(x2[:], x2[:], b2t[:], op=add)
                nc.vector.tensor_tensor(x2[:], x2[:], x1_tiles[g][:], op=amax)
                pt = psum.tile([128, 128], f32, tag="ps")
                nc.tensor.transpose(pt[:], x2[:], ident[:])
                xt = epool.tile([128, 128], mdt, tag="xt")
                nc.vector.tensor_copy(xt[:], pt[:])
                po = psum.tile([128, FOUT], f32, tag="ps")
                nc.tensor.matmul(po[:], lhsT=xt[:], rhs=wf[:], start=True,
                                 stop=True)
                ob = epool.tile([128, FOUT], f32, tag="ob")
                nc.vector.tensor_tensor(ob[:], po[:], bft[:], op=add)
                nc.sync.dma_start(out_d[g * 128:(g + 1) * 128, :], ob[:])

            edge_phase(h2f_d, epilogue2)

    nc.compile()
    return nc


# ------------------------------- host driver -------------------------------

def make_in_maps(cfg, sched, inputs, dis_pad, idx_all, dloc_all):
    node_feat = np.asarray(inputs["node_feat"], np.float32)
    W1 = np.asarray(inputs["W1"], np.float32)
    b1 = np.asarray(inputs["b1"], np.float32)
    gamma1 = np.asarray(inputs["gamma1"], np.float32)
    beta1 = np.asarray(inputs["beta1"], np.float32)
    mean1 = np.asarray(inputs["mean1"], np.float32)
    var1 = np.asarray(inputs["var1"], np.float32)
    W2 = np.asarray(inputs["W2"], np.float32)
    b2 = np.asarray(inputs["b2"], np.float32)
    Wf = np.asarray(inputs["Wf"], np.float32)
    bf = np.asarray(inputs["bf"], np.float32)

    A = gamma1 / np.sqrt(var1 + BN_EPS)
    W1A = (W1 * A[None, :]).astype(np.float32)
    D1 = (b1 * A + beta1 - mean1 * A).astype(np.float32)

    npad = cfg.npad
    nf_pad = np.zeros((npad, FIN), np.float32)
    nf_pad[:cfg.n] = node_feat
    nf_pad *= dis_pad[:, None]          # fold dis_u into the gather table
    nfT = np.ascontiguousarray(nf_pad.T)                     # [128, npad]
    iota = np.tile(np.arange(128, dtype=np.float32), (128, 1))
    if cfg.msg_bf16:
        import ml_dtypes
        mdt_np = ml_dtypes.bfloat16
    else:
        mdt_np = np.float32

    common = {
        "nfT": nfT.astype(mdt_np),
        "w1a": W1A.astype(mdt_np),
        "w2": W2.astype(mdt_np),
        "wf": Wf.astype(mdt_np),
        "d1t": np.tile(D1, (128, 1)),
        "b2t": np.tile(b2, (128, 1)),
        "bft": np.tile(bf, (128, 1)),
        "iota": iota.astype(mdt_np),
        "ident": np.eye(128, dtype=np.float32),
    }
    in_maps = []
    for c in range(cfg.ncores):
        lo = c * cfg.shard
        diss = dis_pad[lo:lo + cfg.shard].reshape(-1, 128).T
        in_maps.append(dict(
            common,
            idxs=idx_all[c],
            dloc=dloc_all[c].astype(mdt_np),
            diss=np.ascontiguousarray(diss),
        ))
    return in_maps


def run(cfg, inputs, trace=False, verbose=False):
    """Full pipeline: prep -> build -> execute on 8 cores -> assemble."""
    import time
    from concourse.bass_utils import run_bass_kernel_spmd
    from concourse.bass_interp import get_hw_module

    t0 = time.time()
    sched, dis_pad, idx_all, dloc_all = prepare(cfg, inputs["edge_index"])
    if verbose:
        print(f"[prep {time.time()-t0:.1f}s] tiles={sched.tot_tiles} "
              f"slots={sched.tot_slots}", flush=True)
    t0 = time.time()
    nc = build_module(cfg, sched)
    if verbose:
        print(f"[build+compile {time.time()-t0:.1f}s]", flush=True)
    t0 = time.time()
    in_maps = make_in_maps(cfg, sched, inputs, dis_pad, idx_all, dloc_all)
    nc.m = get_hw_module(nc.m)
    res = run_bass_kernel_spmd(nc, in_maps, core_ids=list(range(cfg.ncores)),
                               trace=trace)
    if verbose:
        print(f"[execute {time.time()-t0:.1f}s]", flush=True)
    out = np.concatenate([r["out"] for r in res.results], axis=0)[:cfg.n]
    return np.asarray(out, np.float32), res


def kernel(**inputs) -> np.ndarray:
    out, _ = run(CFG, inputs, trace=False)
    return out



# revision 15
# speedup vs baseline: 2.0553x; 1.0073x over previous
"""GCN+JumpingKnowledge inference kernel for Trainium2 (8 NeuronCores).

Computation (matches PyG GCNConv defaults, eval mode):
    deg[v]  = in_degree(v) + 1  (self loops)
    dis     = deg ** -0.5
    agg(x)  = for each v: sum over edges (u->v) incl self loop of
              dis[u]*dis[v] * (x @ W)[u]
    x1 = relu(BN(agg1(node_feat) + b1))
    x2 = agg2(x1) + b2
    out = max(x1, x2) @ Wf + bf

Distribution strategy (8 cores):
  * Destination nodes sharded contiguously across cores (12544 per core,
    graph padded to 100352 nodes with isolated nodes).
  * Layer-1 dense transform h1' = dis * (node_feat @ W1*A) is computed
    replicated on every core (cheaper than an AllGather of it).
  * Layer-2 table h2' = dis * (x1 @ W2) is computed per-shard and
    exchanged with a single AllGather collective.
  * Edge aggregation: edges sorted by (dest-group, source-chunk); source
    rows fetched with dma_gather (int16 indices -> 4 source chunks of
    <=32768 rows); per 128-edge tile a selection matrix S[e,d] =
    (dloc[e] == d) is built on DVE/ACT and the segment sum is done as
    S.T @ msg on the PE into one PSUM bank per 128-dest group.
  * Self loops are appended to the edge list (dis_v * h'_v term).
"""

import math
import os
from contextlib import ExitStack

import numpy as np

# ---------------- problem constants (hardcoded by contract) ----------------
N = 100000
E = 1600000
FIN = 128
HID = 128
FOUT = 40
BN_EPS = 1e-5
NCORES = 8


class Config:
    """Geometry of the kernel; small configs used for simulator tests."""

    def __init__(self, n=N, ncores=NCORES, groups_per_core=98, wave=6,
                 chunk=32768, msg_bf16=True):
        self.n = n
        self.ncores = ncores
        self.G = groups_per_core          # 128-dest groups per core
        self.shard = 128 * groups_per_core
        self.npad = self.shard * ncores
        self.wave = wave                  # groups aggregated per psum wave
        self.chunk = chunk                # max rows addressable by int16 idx
        self.msg_bf16 = msg_bf16
        # chunk layout: assignment ranges + gather base rows.  Row u is
        # assigned to range [starts[i], starts[i+1]); its gather index is
        # u - bases[i], which must stay in [0, chunk).
        starts = []
        bases = []
        s = 0
        while True:
            starts.append(s)
            if self.npad - s <= chunk:           # final chunk covers tail
                bases.append(max(0, min(s, self.npad - chunk)))
                break
            bases.append(s)
            s += chunk
        self.chunk_starts = starts                # range starts
        self.chunk_bases = bases                  # gather AP base row
        self.nchunks = len(starts)
        self.chunk_rows = [min(chunk, self.npad - b) for b in bases]

    def chunk_of(self, u):
        """Chunk id for each (array of) source row id."""
        return np.minimum(np.searchsorted(self.chunk_starts, u, side="right")
                          - 1, self.nchunks - 1)


CFG = Config()


# ---------------------------- host preprocessing ---------------------------

class Sched:
    """Program schedule shared by every core (max over per-core needs).

    Slot layout: per (wave, chunk) segment, the runs of the wave's groups
    are laid out back to back, each sized to the cross-core max count
    M[g,k]; only the segment total is rounded up to 128.  A 128-slot tile
    may therefore span several groups; each (tile, group) pair becomes a
    "subtile" with its own masked dloc column and matmul.
    """

    def __init__(self, cfg, M):
        self.cfg = cfg
        self.M = M  # [G, nchunks] max slot count per (group, chunk)
        g_per_wave = cfg.wave
        self.waves = [list(range(w, min(w + g_per_wave, cfg.G)))
                      for w in range(0, cfg.G, g_per_wave)]
        # segments in program order
        self.segs = []      # (w, k, slot_off, n_slots, subtiles)
        # run_bounds[(g, k)] = (abs_lo, abs_hi) slot interval of the run
        self.run_bounds = {}
        self.tot_slots = 0
        self.tot_tiles = 0
        grp_subtiles = {g: [] for g in range(cfg.G)}
        for w, groups in enumerate(self.waves):
            for k in range(cfg.nchunks):
                seg_used = int(M[groups, k].sum())
                if seg_used == 0:
                    continue
                n_slots = -(-seg_used // 128) * 128
                off = self.tot_slots
                pos = off
                for g in groups:
                    if M[g, k]:
                        self.run_bounds[(g, k)] = (pos, pos + int(M[g, k]))
                        pos += int(M[g, k])
                subtiles = []
                for lt in range(n_slots // 128):
                    t_lo, t_hi = off + lt * 128, off + (lt + 1) * 128
                    for g in groups:
                        b = self.run_bounds.get((g, k))
                        if b and b[0] < t_hi and b[1] > t_lo:
                            subtiles.append([lt, g, False, False])
                            grp_subtiles[g].append(subtiles[-1])
                self.segs.append((w, k, off, n_slots, subtiles))
                self.tot_slots += n_slots
                self.tot_tiles += len(subtiles)
        for g in range(cfg.G):
            sts = grp_subtiles[g]
            assert sts, f"group {g} has no subtiles"
            sts[0][2] = True     # first
            sts[-1][3] = True    # last


def _edge_buckets(cfg, row, col, core):
    """Per-core edge lists (with self loops) keyed by (group, chunk)."""
    lo, hi = core * cfg.shard, (core + 1) * cfg.shard
    m = (col >= lo) & (col < hi)
    r, c = row[m], col[m]
    # self loops for real nodes of this shard
    sl = np.arange(lo, min(hi, cfg.n), dtype=np.int64)
    r = np.concatenate([r, sl])
    c = np.concatenate([c, sl])
    g = (c - lo) >> 7
    k = cfg.chunk_of(r)
    lidx = r - np.asarray(cfg.chunk_bases, np.int64)[k]
    dloc = (c - lo) & 127
    order = np.lexsort((k, g))
    return g[order], k[order], lidx[order], dloc[order]


def prepare(cfg, edge_index):
    """Build the shared schedule + per-core idx/dloc streams."""
    row = np.asarray(edge_index[0], np.int64)
    col = np.asarray(edge_index[1], np.int64)
    deg = np.bincount(col, minlength=cfg.n).astype(np.float32) + 1.0
    dis = 1.0 / np.sqrt(deg)
    dis_pad = np.zeros(cfg.npad, np.float32)
    dis_pad[:cfg.n] = dis

    per_core = [_edge_buckets(cfg, row, col, c) for c in range(cfg.ncores)]
    counts = np.zeros((cfg.ncores, cfg.G, cfg.nchunks), np.int64)
    for c, (g, k, _, _) in enumerate(per_core):
        np.add.at(counts[c], (g, k), 1)
    M = counts.max(axis=0)  # [G, nchunks] run lengths
    sched = Sched(cfg, M)

    # slot -> owning group map (uniform across cores)
    slot_group = np.full(sched.tot_slots, -1, np.int64)
    for (g, k), (lo, hi) in sched.run_bounds.items():
        slot_group[lo:hi] = g

    idx_all = []
    dloc_all = []
    for c in range(cfg.ncores):
        g, k, lidx, dloc = per_core[c]
        slots_idx = np.zeros(sched.tot_slots, np.int32)
        slots_dloc = np.full(sched.tot_slots, -1.0, np.float32)
        key = g * cfg.nchunks + k
        bounds = np.searchsorted(key, np.arange(cfg.G * cfg.nchunks + 1))
        for (gg, kk), (lo, hi) in sched.run_bounds.items():
            b0, b1 = bounds[gg * cfg.nchunks + kk], bounds[gg * cfg.nchunks + kk + 1]
            n = b1 - b0
            assert n <= hi - lo
            slots_idx[lo:lo + n] = lidx[b0:b1]
            slots_dloc[lo:lo + n] = dloc[b0:b1]
        # wrap idx per segment: position i -> [i%16, i//16], tiled to 128 rows
        idx_w = np.zeros((128, sched.tot_slots // 16), np.int16)
        for w, kk, seg_off, n_slots, subtiles in sched.segs:
            seg = slots_idx[seg_off:seg_off + n_slots]
            wrapped = seg.reshape(-1, 16).T.astype(np.int16)   # [16, n/16]
            idx_w[:, seg_off // 16:(seg_off + n_slots) // 16] = np.tile(
                wrapped, (8, 1))
        # per-subtile dloc columns, masked to the subtile's group
        dloc_w = np.full((128, sched.tot_tiles), -1.0, np.float32)
        tid = 0
        for w, kk, seg_off, n_slots, subtiles in sched.segs:
            for lt, gg, first, last in subtiles:
                t_lo = seg_off + lt * 128
                sl = slice(t_lo, t_lo + 128)
                dloc_w[:, tid] = np.where(slot_group[sl] == gg,
                                          slots_dloc[sl], -1.0)
                tid += 1
        idx_all.append(idx_w)
        dloc_all.append(np.ascontiguousarray(dloc_w))
    return sched, dis_pad, idx_all, dloc_all


# ------------------------------- bass builder ------------------------------

def build_module(cfg, sched, reps=1, single=False, ag_copy=False):
    import concourse.bacc as bacc
    import concourse.tile as tile
    from concourse import mybir
    import concourse.bass as bass

    f32 = mybir.dt.float32
    mdt = mybir.dt.bfloat16 if cfg.msg_bf16 else f32
    i16 = mybir.dt.int16
    eq = mybir.AluOpType.is_equal
    mult = mybir.AluOpType.mult
    add = mybir.AluOpType.add
    amax = mybir.AluOpType.max
    Act = mybir.ActivationFunctionType

    nc = bacc.Bacc("TRN2", target_bir_lowering=False, debug=False,
                   num_devices=1 if single else cfg.ncores,
                   num_swdge_queues=4)
    npad, shard, G = cfg.npad, cfg.shard, cfg.G
    ntiles_all = npad // 128
    tot16 = sched.tot_slots // 16
    TT = sched.tot_tiles

    # --- I/O ---
    ein, eout = "ExternalInput", "ExternalOutput"
    nfT_d = nc.dram_tensor("nfT", [128, npad], mdt, kind=ein)
    idx_d = nc.dram_tensor("idxs", [128, tot16], i16, kind=ein)
    dloc_d = nc.dram_tensor("dloc", [128, TT], mdt, kind=ein)
    diss_d = nc.dram_tensor("diss", [128, G], f32, kind=ein)
    w1a_d = nc.dram_tensor("w1a", [FIN, HID], mdt, kind=ein)
    w2_d = nc.dram_tensor("w2", [HID, HID], mdt, kind=ein)
    wf_d = nc.dram_tensor("wf", [HID, FOUT], mdt, kind=ein)
    d1_d = nc.dram_tensor("d1t", [128, HID], f32, kind=ein)
    b2_d = nc.dram_tensor("b2t", [128, HID], f32, kind=ein)
    bf_d = nc.dram_tensor("bft", [128, FOUT], f32, kind=ein)
    iota_d = nc.dram_tensor("iota", [128, 128], mdt, kind=ein)
    ident_d = nc.dram_tensor("ident", [128, 128], f32, kind=ein)
    out_d = nc.dram_tensor("out", [shard, FOUT], f32, kind=eout)
    # internal
    h1_d = nc.dram_tensor("h1full", [npad, HID], mdt)
    h2s_d = nc.dram_tensor("h2shard", [shard, HID], mdt)
    h2f_d = nc.dram_tensor("h2full", [npad, HID], mdt, addr_space="Shared")

    with tile.TileContext(nc) as tc, ExitStack() as ctx:
        from concourse.library_config import mlp as mlp_lib
        nc.gpsimd.load_library(mlp_lib)

        consts = ctx.enter_context(tc.tile_pool(name="consts", bufs=1))
        psum = ctx.enter_context(tc.tile_pool(name="psum", bufs=8,
                                              space="PSUM"))
        xpool = ctx.enter_context(tc.tile_pool(name="x", bufs=4))
        hpool = ctx.enter_context(tc.tile_pool(name="h", bufs=4))
        idxp = ctx.enter_context(tc.tile_pool(name="idx", bufs=6))
        msgp = ctx.enter_context(tc.tile_pool(name="msg", bufs=6))
        spool = ctx.enter_context(tc.tile_pool(name="s", bufs=4))
        x1pool = ctx.enter_context(tc.tile_pool(name="x1", bufs=G))
        epool = ctx.enter_context(tc.tile_pool(name="ep", bufs=3))

        def load_const(dram, shape, dtype):
            t = consts.tile(shape, dtype, tag=dram.name)
            nc.sync.dma_start(t[:], dram.ap())
            return t

        w1a = load_const(w1a_d, [FIN, HID], mdt)
        w2 = load_const(w2_d, [HID, HID], mdt)
        wf = load_const(wf_d, [HID, FOUT], mdt)
        d1t = load_const(d1_d, [128, HID], f32)
        b2t = load_const(b2_d, [128, HID], f32)
        bft = load_const(bf_d, [128, FOUT], f32)
        iota = load_const(iota_d, [128, 128], mdt)
        ident = load_const(ident_d, [128, 128], f32)
        diss = load_const(diss_d, [128, G], f32)
        dloc = load_const(dloc_d, [128, TT], mdt)

        for rep in range(reps):
            # ---------------- phase A: h1' for all nodes (replicated) ----------
            # nfT comes pre-scaled by dis from the host, so h1' is a plain
            # matmul.  Process 1024 nodes per DMA, 512 per psum bank.
            BW = 1024
            for j0 in range(0, ntiles_all * 128, BW):
                bw = min(BW, ntiles_all * 128 - j0)
                nf = xpool.tile([128, BW], mdt, tag="nf")
                nc.sync.dma_start(nf[:, :bw], nfT_d[:, j0:j0 + bw])
                for s0 in range(0, bw, 512):
                    sw = min(512, bw - s0)
                    ps = psum.tile([128, 512], f32, tag="ps")
                    for m0 in range(0, sw, 128):
                        nc.tensor.matmul(
                            ps[:, m0:m0 + 128],
                            lhsT=nf[:, s0 + m0:s0 + m0 + 128],
                            rhs=w1a[:], start=True, stop=True)
                    h = hpool.tile([128, 4, HID], mdt, tag="h")
                    nc.scalar.activation(h[:, :sw // 128, :], ps[:, :sw],
                                         Act.Copy)
                    v0 = j0 + s0
                    nc.sync.dma_start(
                        h1_d[v0:v0 + sw, :].rearrange(
                            "(j p) f -> p j f", p=128),
                        h[:, :sw // 128, :])

            x1_tiles = [x1pool.tile([128, HID], f32, tag="x1", name=f"x1_{rep}_{g}")
                        for g in range(G)]

            # ---------------- edge aggregation -------------------------------
            qctr = [0]

            def edge_phase(table_d, epilogue):
                tid0 = [0]
                for w, groups in enumerate(sched.waves):
                    gps = {g: psum.tile([128, HID], f32, tag="ps",
                                        name=f"agg_{rep}_w{w}_g{g}")
                           for g in groups}
                    wave_segs = [s for s in sched.segs if s[0] == w]
                    w_off = wave_segs[0][2]
                    w_slots = sum(s[3] for s in wave_segs)
                    idx_sb = idxp.tile([128, w_slots // 16], i16, tag="idx")
                    nc.sync.dma_start(
                        idx_sb[:],
                        idx_d[:, w_off // 16:(w_off + w_slots) // 16])
                    for (sw, k, seg_off, n_slots, subtiles) in wave_segs:
                        so16 = (seg_off - w_off) // 16
                        J = n_slots // 128
                        msg = msgp.tile([128, J, 128], mdt, tag="msg")
                        base = cfg.chunk_bases[k]
                        # HW SWDGE descriptor ring caps one gather at 1024
                        # descriptors; cycle the 4 SWDGE queues so descriptor
                        # generation runs on all 4 Q7 core pairs in parallel.
                        GMAX = 1024
                        for sub in range(0, n_slots, GMAX):
                            ns = min(GMAX, n_slots - sub)
                            nc.gpsimd.dma_gather(
                                out_ap=msg[:, sub // 128:(sub + ns) // 128, :],
                                in_ap=table_d[base:base + cfg.chunk_rows[k], :],
                                idxs_ap=idx_sb[:, so16 + sub // 16:
                                               so16 + (sub + ns) // 16],
                                num_idxs=ns,
                                num_idxs_reg=ns,
                                elem_size=HID,
                                queue_num=qctr[0] % 4,
                            )
                            qctr[0] += 1
                        # batched S build: S[e, d] = (dloc[e] == iota[d]) for
                        # the segment's subtiles; two halves so the first
                        # half's matmuls start before the whole batch builds.
                        nst = len(subtiles)
                        sb = spool.tile([128, nst, 128], mdt, tag="s",
                                        name=f"s_{rep}_{table_d.name}"
                                             f"_w{w}_k{k}")
                        h0 = (nst + 1) // 2
                        for lo, hi in ((0, h0), (h0, nst)):
                            if hi == lo:
                                continue
                            nb = hi - lo
                            io_b = iota[:].unsqueeze(1).broadcast_to(
                                [128, nb, 128])
                            dl_b = (dloc[:, tid0[0] + lo:tid0[0] + hi]
                                    .unsqueeze(2)
                                    .broadcast_to([128, nb, 128]))
                            nc.vector.tensor_tensor(sb[:, lo:hi, :],
                                                    io_b, dl_b, op=eq)
                        tid0[0] += nst
                        for sj, (lt, g, first, last) in enumerate(subtiles):
                            nc.tensor.matmul(gps[g][:], lhsT=sb[:, sj, :],
                                             rhs=msg[:, lt, :],
                                             start=first, stop=last)
                    for g in groups:
                        epilogue(g, gps[g])

            # ---------------- layer 1 ----------------------------------------
            def epilogue1(g, ps):
                x1 = x1_tiles[g]
                # x1 = relu(dis*psum + D1)
                nc.vector.tensor_scalar(x1[:], ps[:], diss[:, g:g + 1], None,
                                        op0=mult)
                nc.vector.tensor_tensor(x1[:], x1[:], d1t[:], op=add)
                nc.scalar.activation(x1[:], x1[:], Act.Relu)
                # h2' = dis * (x1 @ W2); via PE transpose of x1
                pt = psum.tile([128, 128], f32, tag="ps")
                nc.tensor.transpose(pt[:], x1[:], ident[:])
                x1t = epool.tile([128, 128], mdt, tag="x1t")
                nc.vector.tensor_copy(x1t[:], pt[:])
                ph = psum.tile([128, HID], f32, tag="ps")
                nc.tensor.matmul(ph[:], lhsT=x1t[:], rhs=w2[:], start=True,
                                 stop=True)
                h2 = epool.tile([128, HID], mdt, tag="h2")
                nc.scalar.activation(h2[:], ph[:], Act.Copy,
                                     scale=diss[:, g:g + 1])
                nc.sync.dma_start(h2s_d[g * 128:(g + 1) * 128, :], h2[:])

            edge_phase(h1_d, epilogue1)

            # ---------------- exchange layer-2 table -------------------------
            if single or ag_copy:
                # stand-in for the AllGather so TimelineSim (single core,
                # no collectives) can run; timing of the real AG is separate
                nc.sync.dma_start(h2f_d[0:shard, :], h2s_d.ap())
            else:
                nc.gpsimd.collective_compute(
                    "AllGather",
                    mybir.AluOpType.bypass,
                    replica_groups=[list(range(cfg.ncores))],
                    ins=[h2s_d.ap()],
                    outs=[h2f_d.ap()],
                )

            # ---------------- layer 2 + JK + final ---------------------------
            def epilogue2(g, ps):
                x2 = epool.tile([128, HID], f32, tag="x2")
                nc.vector.tensor_scalar(x2[:], ps[:], diss[:, g:g + 1], None,
                                        op0=mult)
                nc.vector.tensor_tensor(x2[:], x2[:], b2t[:], op=add)
                nc.vector.tensor_tensor(x2[:], x2[:], x1_tiles[g][:], op=amax)
                pt = psum.tile([128, 128], f32, tag="ps")
                nc.tensor.transpose(pt[:], x2[:], ident[:])
                xt = epool.tile([128, 128], mdt, tag="xt")
                nc.vector.tensor_copy(xt[:], pt[:])
                po = psum.tile([128, FOUT], f32, tag="ps")
                nc.tensor.matmul(po[:], lhsT=xt[:], rhs=wf[:], start=True,
                                 stop=True)
                ob = epool.tile([128, FOUT], f32, tag="ob")
                nc.vector.tensor_tensor(ob[:], po[:], bft[:], op=add)
                nc.sync.dma_start(out_d[g * 128:(g + 1) * 128, :], ob[:])

            edge_phase(h2f_d, epilogue2)

    nc.compile()
    return nc


# ------------------------------- host driver -------------------------------

def make_in_maps(cfg, sched, inputs, dis_pad, idx_all, dloc_all):
    node_feat = np.asarray(inputs["node_feat"], np.float32)
    W1 = np.asarray(inputs["W1"], np.float32)
    b1 = np.asarray(inputs["b1"], np.float32)
    gamma1 = np.asarray(inputs["gamma1"], np.float32)
    beta1 = np.asarray(inputs["beta1"], np.float32)
    mean1 = np.asarray(inputs["mean1"], np.float32)
    var1 = np.asarray(inputs["var1"], np.float32)
    W2 = np.asarray(inputs["W2"], np.float32)
    b2 = np.asarray(inputs["b2"], np.float32)
    Wf = np.asarray(inputs["Wf"], np.float32)
    bf = np.asarray(inputs["bf"], np.float32)

    A = gamma1 / np.sqrt(var1 + BN_EPS)
    W1A = (W1 * A[None, :]).astype(np.float32)
    D1 = (b1 * A + beta1 - mean1 * A).astype(np.float32)

    npad = cfg.npad
    nf_pad = np.zeros((npad, FIN), np.float32)
    nf_pad[:cfg.n] = node_feat
    nf_pad *= dis_pad[:, None]          # fold dis_u into the gather table
    nfT = np.ascontiguousarray(nf_pad.T)                     # [128, npad]
    iota = np.tile(np.arange(128, dtype=np.float32), (128, 1))
    if cfg.msg_bf16:
        import ml_dtypes
        mdt_np = ml_dtypes.bfloat16
    else:
        mdt_np = np.float32

    common = {
        "nfT": nfT.astype(mdt_np),
        "w1a": W1A.astype(mdt_np),
        "w2": W2.astype(mdt_np),
        "wf": Wf.astype(mdt_np),
        "d1t": np.tile(D1, (128, 1)),
        "b2t": np.tile(b2, (128, 1)),
        "bft": np.tile(bf, (128, 1)),
        "iota": iota.astype(mdt_np),
        "ident": np.eye(128, dtype=np.float32),
    }
    in_maps = []
    for c in range(cfg.ncores):
        lo = c * cfg.shard
        diss = dis_pad[lo:lo + cfg.shard].reshape(-1, 128).T
        in_maps.append(dict(
            common,
            idxs=idx_all[c],
            dloc=dloc_all[c].astype(mdt_np),
            diss=np.ascontiguousarray(diss),
        ))
    return in_maps


def run(cfg, inputs, trace=False, verbose=False):
    """Full pipeline: prep -> build -> execute on 8 cores -> assemble."""
    import time
    from concourse.bass_utils import run_bass_kernel_spmd
    from concourse.bass_interp import get_hw_module

    t0 = time.time()
    sched, dis_pad, idx_all, dloc_all = prepare(cfg, inputs["edge_index"])
    if verbose:
        print(f"[prep {time.time()-t0:.1f}s] tiles={sched.tot_tiles} "
              f"slots={sched.tot_slots}", flush=True)
    t0 = time.time()
    nc = build_module(cfg, sched)
    if verbose:
        print(f"[build+compile {time.time()-t0:.1f}s]", flush=True)
    t0 = time.time()
    in_maps = make_in_maps(cfg, sched, inputs, dis_pad, idx_all, dloc_all)
    nc.m = get_hw_module(nc.m)
    res = run_bass_kernel_spmd(nc, in_maps, core_ids=list(range(cfg.ncores)),
                               trace=trace)
    if verbose:
        print(f"[execute {time.time()-t0:.1f}s]", flush=True)
    out = np.concatenate([r["out"] for r in res.results], axis=0)[:cfg.n]
    return np.asarray(out, np.float32), res


def kernel(**inputs) -> np.ndarray:
    out, _ = run(CFG, inputs, trace=False)
    return out

